# revision 23
# baseline (speedup 1.0000x reference)
"""Trainium2 Bass kernel for nn_EnsembleModel (histogram_binning).

Math:
  hist[p,q]  = sum_{b,i,j} [adds[b,i]==p] * a_arc[b,i,j] * [adds[b,j]==q]
  score      = sigmoid(hist)                                  # [50,50]
  out[b,i,j] = s_arc[b,i,j] + ALPHA * score[pos[b,i], pos[b,j]]

Histogram and gather-broadcast are TensorEngine matmuls against one-hot
matrices (U = onehot(adds), VT = onehot(pos).T) prepared host-side in
partition-major layout:

  phase 1 (per batch):  P[p,j]  = sum_i U[i,p] A[i,j]   (lhsT=U, rhs=A)
                        PT      = PE-transpose of P (128-chunks)
                        hist   += PT.T @ U              (lhsT=PT, rhs=U)
  AllGather(partials) -> 8-way tree sum -> sigmoid(scale*h)*ALPHA -> sc bf16
  phase 2 (per batch):  GT[q,i] = sum_p sc[p,q] VT[p,i] (lhsT=sc, rhs=VT)
                        out     = s_arc + GT.T @ VT     (lhsT=GT slice)

Key performance facts this schedule is built around:
  - The PE has p-states: 1.2GHz normally, 2.4GHz only after ~3us of
    continuous execution; idle gaps reset the ramp. So the PE instruction
    stream is kept dense: per batch-slot we issue P-matmuls(b),
    transposes(b-1), hist-matmuls(b-2) -- a 2-deep software pipeline that
    gives the DVE/ACT a full slot to drain the PSUM->SBUF copies the next
    PE stage needs.
  - AllGather + on-chip tree sum instead of AllReduce (AR has a ~30us
    floor here; AG is shorter and the 8-way sum is 3 DVE adds).
  - Phase 2's per-chunk finalize (s + gathered score) is split across
    engines: some chunks are a direct DVE add from f32 PSUM, most are an
    ACT PSUM->SBUF copy followed by an all-bf16 DVE add (2 elem/cycle).
  - Ring discipline (engine queues are in-order, ~0.7us per dma_start on
    the issuing engine): sync = a-loads, then s-loads (FIFO serializes
    them), then half the stores; scalar = consts + gathered-hist load +
    ACT copies; gpsimd = cc bounce + last s-load + other half of stores.
  - a[0] is loaded in 128-row chunks: concurrent DMAs on one ring complete
    together, so a monolithic first load would delay the first matmul.

Data-parallel over batch: 8 batches per core on 8 NeuronCores.
"""

import numpy as np
import ml_dtypes

ALPHA = 0.3
NP = 50          # n_pos
SL = 1024        # sequence length
BZ = 64          # global batch
NCORES = 8
B = BZ // NCORES  # local batch per core
NCH = SL // 128   # 128-row chunks per matrix
NBLK = SL // 512  # 512-col blocks per matrix
_CACHE = {}


def _build_nc():
    import concourse.bacc as bacc
    import concourse.mybir as mybir
    import concourse.tile as tile
    from concourse.tile import add_dep_helper

    f32 = mybir.dt.float32
    bf16 = mybir.dt.bfloat16
    nc = bacc.Bacc(
        "TRN2", target_bir_lowering=False, debug=False, num_devices=NCORES
    )

    a_d = nc.dram_tensor("a", [B, 128, NCH, SL], bf16, kind="ExternalInput")
    s_d = nc.dram_tensor("s", [B, 128, NCH, SL], bf16, kind="ExternalInput")
    u_d = nc.dram_tensor("u", [128, B, NCH, NP], bf16, kind="ExternalInput")
    vt_d = nc.dram_tensor("vt", [NP, B, SL], bf16, kind="ExternalInput")
    eye_d = nc.dram_tensor("eye", [NP, NP], bf16, kind="ExternalInput")
    out_d = nc.dram_tensor("out", [B, SL, SL], bf16, kind="ExternalOutput")

    with tile.TileContext(nc) as tc:
        with (
            tc.tile_pool(name="const", bufs=1) as const_pool,
            tc.tile_pool(name="abfpool", bufs=2) as abf_pool,
            tc.tile_pool(name="spool", bufs=7) as s_pool,
            tc.tile_pool(name="opool", bufs=6) as o_pool,
            tc.tile_pool(name="ppool", bufs=2) as p_pool,
            tc.tile_pool(name="ptsb", bufs=16) as pt_pool,
            tc.tile_pool(name="gtsb", bufs=2) as gt_pool,
            tc.tile_pool(name="small", bufs=1) as small_pool,
            tc.tile_pool(name="dram", bufs=1, space="DRAM") as dram_pool,
        ):
            # Persistent operands (scalar/ACT ring; small, land in ~3us).
            u_sb = const_pool.tile([128, B, NCH, NP], bf16)
            eye_sb = const_pool.tile([NP, NP], bf16)
            vt_sb = const_pool.tile([NP, B, SL], bf16)
            # u first: it is the only const the first P-matmul needs.
            nc.scalar.dma_start(u_sb[:], u_d[:])
            nc.scalar.dma_start(eye_sb[:], eye_d[:])
            nc.scalar.dma_start(vt_sb[:], vt_d[:])

            # a: HWDGE loads on the sync ring. Concurrent DMAs on a ring
            # interleave at packet granularity, so a monolithic a[0] would
            # complete only when a[1],a[2] do too -- split a[0] into chunk
            # loads so the first P-matmuls start as early as possible.
            abf_tiles = []
            a_loads = []
            for b in range(B):
                at = abf_pool.tile([128, NCH, SL], bf16, tag="abf")
                if b == 0:
                    for c in range(NCH):
                        ld = nc.sync.dma_start(at[:, c, :], a_d[b, :, c, :])
                else:
                    ld = nc.sync.dma_start(at[:], a_d[b])
                abf_tiles.append(at)
                a_loads.append(ld)

            # s[0..5]: sync HWDGE ring; first gated on last a-cast so a has
            # the full HBM bandwidth while it streams. s[6..7] recycle pool
            # slots, so they go on the gpsimd queue AFTER the collective --
            # their pool-free waits must not block store issue on sync.
            s_tiles = []
            for b in range(7):
                st = s_pool.tile([128, NCH, SL], bf16, tag="s")
                sld = nc.sync.dma_start(st[:], s_d[b])
                if b == 0:
                    add_dep_helper(
                        sld.ins, a_loads[-1].ins,
                        reason="s-loads after a-loads: a is latency-critical",
                    )
                s_tiles.append(st)

            # ---- Phase 1: dense-PE pipelined partial histogram ----
            with (
                tc.tile_pool(name="histps", bufs=1, space="PSUM") as hist_pool,
                tc.tile_pool(name="pps", bufs=3, space="PSUM") as pps_pool,
                tc.tile_pool(name="tpps", bufs=3, space="PSUM") as tpps_pool,
            ):
                hist_ps = hist_pool.tile([NP, NP], f32)
                p_sbs = [None] * B
                tp_bigs = [None] * B
                pts_tiles = [[None] * NCH for _ in range(B)]
                for slot in range(B + 2):
                    # PE stage 1: P-matmuls for batch `slot`
                    if slot < B:
                        b = slot
                        p_sb = p_pool.tile([NP, SL], bf16, tag="p")
                        p_sbs[b] = p_sb
                        for jb in range(NBLK):
                            p_ps = pps_pool.tile([NP, 512], f32, tag="pp")
                            for ic in range(NCH):
                                nc.tensor.matmul(
                                    p_ps[:],
                                    u_sb[:, b, ic, :],
                                    abf_tiles[b][:, ic, jb * 512:(jb + 1) * 512],
                                    start=(ic == 0),
                                    stop=(ic == NCH - 1),
                                )
                            nc.vector.tensor_copy(
                                p_sb[:, jb * 512:(jb + 1) * 512], p_ps[:]
                            )
                    # PE stage 2: transposes for batch slot-1 (dense; the
                    # PSUM->SBUF pts copies drain during the next P block)
                    if 1 <= slot <= B:
                        b = slot - 1
                        tp_big = tpps_pool.tile([128, NCH, NP], bf16, tag="tp")
                        tp_bigs[b] = tp_big
                        for jc in range(NCH):
                            nc.tensor.transpose(
                                tp_big[:, jc, :],
                                p_sbs[b][:, jc * 128:(jc + 1) * 128],
                                eye_sb[:],
                            )
                        for jc in range(NCH):
                            pts = pt_pool.tile([128, NP], bf16, tag="pts")
                            pts_tiles[b][jc] = pts
                            eng = nc.vector if jc % 2 == 0 else nc.scalar
                            if eng is nc.vector:
                                eng.tensor_copy(pts[:], tp_big[:, jc, :])
                            else:
                                eng.activation(
                                    pts[:], tp_big[:, jc, :],
                                    mybir.ActivationFunctionType.Copy,
                                )
                    # PE stage 3: hist accumulation for batch slot-2
                    if slot >= 2:
                        b = slot - 2
                        for jc in range(NCH):
                            nc.tensor.matmul(
                                hist_ps[:],
                                pts_tiles[b][jc][:],
                                u_sb[:, b, jc, :],
                                start=(b == 0 and jc == 0),
                                stop=(b == B - 1 and jc == NCH - 1),
                            )
                hist_sb = small_pool.tile([NP, NP], f32, tag="h0")
                nc.vector.tensor_copy(hist_sb[:], hist_ps[:])

            # ---- AllGather partials + tree sum + sigmoid ----
            cc_in = dram_pool.tile([NP, NP], f32, tag="ccin")
            cc_out = dram_pool.tile([NCORES, NP, NP], f32, tag="ccout")
            nc.gpsimd.dma_start(cc_in[:], hist_sb[:])
            nc.gpsimd.collective_compute(
                "AllGather",
                mybir.AluOpType.bypass,
                replica_groups=[list(range(NCORES))],
                ins=[cc_in.opt()],
                outs=[cc_out.opt()],
            )
            # Last s-load recycles a pool slot freed by phase-2 batch-0
            # consumption; it sits first on the (otherwise idle) gpsimd queue
            # so its pool-free wait cannot block store issue.
            for b in range(7, B):
                st = s_pool.tile([128, NCH, SL], bf16, tag="s")
                nc.gpsimd.dma_start(st[:], s_d[b])
                s_tiles.append(st)
            hist8 = small_pool.tile([NP, NCORES, NP], f32, tag="h8")
            nc.scalar.dma_start(hist8[:], cc_out[:].transpose([1, 0, 2]))
            h4 = small_pool.tile([NP, 4, NP], f32, tag="h4")
            nc.vector.tensor_add(h4[:], hist8[:, 0:4, :], hist8[:, 4:8, :])
            h2 = small_pool.tile([NP, 2, NP], f32, tag="h2")
            nc.vector.tensor_add(h2[:], h4[:, 0:2, :], h4[:, 2:4, :])
            h1 = small_pool.tile([NP, NP], f32, tag="h1")
            nc.vector.tensor_add(h1[:], h2[:, 0, :], h2[:, 1, :])
            sc_f = small_pool.tile([NP, NP], f32, tag="scf")
            nc.scalar.activation(
                sc_f[:], h1[:], mybir.ActivationFunctionType.Sigmoid,
                scale=1.0,
            )
            sc = small_pool.tile([NP, NP], bf16, tag="sc")
            nc.vector.tensor_scalar_mul(sc[:], sc_f[:], ALPHA)

            # ---- Phase 2: broadcast-back + add ----
            # Finalize (s + gathered-score, [128,1024] per chunk) is the
            # phase-2 wall, so it is spread across engines per chunk:
            #   mode A ("dve"): DVE tensor_add(s_bf16, o_ps_f32) directly
            #   mode B ("act+dve"): ACT copies PSUM->SBUF bf16, then DVE adds
            #     all-bf16 (2 elem/cycle path) -- splits the work ACT/DVE.
            # Stores (0.65us of issue time each) go on sync+gpsimd, which are
            # otherwise idle here; scalar's engine time is the ACT copies.
            MODES = ["dve", "act+dve", "act+dve", "act+dve",
                     "dve", "act+dve", "act+dve", "act+dve"]
            STORE_ENG = ["sync", "gpsimd", "sync", "gpsimd",
                         "sync", "gpsimd", "sync", "gpsimd"]
            with (
                tc.tile_pool(name="gtps", bufs=2, space="PSUM") as gtps_pool,
                tc.tile_pool(name="ops", bufs=3, space="PSUM") as ops_pool,
                tc.tile_pool(name="gsb", bufs=3) as g_pool,
            ):
                gt_sbs = [None] * B

                def issue_gt(b):
                    gt_sb = gt_pool.tile([NP, SL], bf16, tag="gt")
                    gt_sbs[b] = gt_sb
                    for ib in range(NBLK):
                        gt_ps = gtps_pool.tile([NP, 512], f32, tag="gtp")
                        nc.tensor.matmul(
                            gt_ps[:],
                            sc[:],
                            vt_sb[:, b, ib * 512:(ib + 1) * 512],
                            start=True,
                            stop=True,
                        )
                        nc.vector.tensor_copy(
                            gt_sb[:, ib * 512:(ib + 1) * 512], gt_ps[:]
                        )

                issue_gt(0)
                for b in range(B):
                    if b + 1 < B:
                        issue_gt(b + 1)
                    for c in range(NCH):
                        mode = MODES[c]
                        ot = o_pool.tile([128, SL], bf16, tag="o")
                        o_ps = ops_pool.tile([128, SL], f32, tag="op")
                        for jb in range(NBLK):
                            jsl = slice(jb * 512, (jb + 1) * 512)
                            nc.tensor.matmul(
                                o_ps[:, jsl],
                                gt_sbs[b][:, c * 128:(c + 1) * 128],
                                vt_sb[:, b, jsl],
                                start=True,
                                stop=True,
                            )
                        if mode == "dve":
                            nc.vector.tensor_add(
                                ot[:], s_tiles[b][:, c, :], o_ps[:]
                            )
                        else:
                            g_sb = g_pool.tile([128, SL], bf16, tag="g")
                            nc.scalar.activation(
                                g_sb[:], o_ps[:],
                                mybir.ActivationFunctionType.Copy,
                            )
                            nc.vector.tensor_add(
                                ot[:], s_tiles[b][:, c, :], g_sb[:]
                            )
                        eng = {"sync": nc.sync, "gpsimd": nc.gpsimd}[
                            STORE_ENG[c]]
                        eng.dma_start(
                            out_d[b, c * 128:(c + 1) * 128, :], ot[:]
                        )

    nc.compile()
    return nc


def _get_nc():
    if "nc" not in _CACHE:
        _CACHE["nc"] = _build_nc()
    return _CACHE["nc"]


def kernel(a_arc, s_arc, adds, pos, n_pos, _trace=False, _return_perf=False):
    from concourse.bass_utils import run_bass_kernel_spmd

    assert int(n_pos) == NP
    a = np.asarray(a_arc, dtype=np.float32)
    s = np.asarray(s_arc, dtype=np.float32)
    adds = np.asarray(adds)
    pos = np.asarray(pos)

    rng = np.arange(NP)
    eye = np.eye(NP, dtype=ml_dtypes.bfloat16)

    a_bf = a.astype(ml_dtypes.bfloat16)
    s_bf = s.astype(ml_dtypes.bfloat16)

    in_maps = []
    for k in range(NCORES):
        sl = slice(k * B, (k + 1) * B)
        adds_sh = adds[sl]
        pos_sh = pos[sl]
        # partition-major relayout: [B, SL, SL] -> [B, 128, NCH, SL]
        a_sh = np.ascontiguousarray(
            a_bf[sl].reshape(B, NCH, 128, SL).transpose(0, 2, 1, 3)
        )
        s_sh = np.ascontiguousarray(
            s_bf[sl].reshape(B, NCH, 128, SL).transpose(0, 2, 1, 3)
        )
        # u[p, b, c, q] = [adds[b, c*128+p] == q]  (partition-major)
        u2 = (
            adds_sh.reshape(B, NCH, 128).transpose(2, 0, 1)[..., None] == rng
        ).astype(ml_dtypes.bfloat16)
        # vt[p, b, i] = [pos[b, i] == p]
        vt2 = (rng[:, None, None] == pos_sh[None, :, :]).astype(
            ml_dtypes.bfloat16
        )
        in_maps.append(
            {
                "a": a_sh,
                "s": s_sh,
                "u": np.ascontiguousarray(u2),
                "vt": np.ascontiguousarray(vt2),
                "eye": eye,
            }
        )

    nc = _get_nc()
    res = run_bass_kernel_spmd(
        nc, in_maps, core_ids=list(range(NCORES)), trace=_trace
    )
    out = np.concatenate([r["out"] for r in res.results], axis=0).astype(np.float32)
    if _return_perf:
        return out, res
    return out
